# revision 31
# baseline (speedup 1.0000x reference)
"""Trainium2 Bass kernel for DecoderMultiHeadAttention (B=2, T=2048, C=768, H=12).

Sharding: 8 cores = 2 batches x 4 head-groups (3 heads each).
Per core: compute K,V projections for its head group, RoPE, causal
flash-style attention with transposed score layout, AllGather of
attention outputs within each batch group of 4 cores, then a
column-sharded output projection.

Note: the reference uses q = rope(v) (faithful source bug), so the
q-chunk of W_att (columns C..2C) is never used and is not computed.
"""

import sys

_REPO = "/opt/trn_rl_repo"
if _REPO not in sys.path:
    sys.path.insert(0, _REPO)

import numpy as np

import concourse.bass as bass
import concourse.mybir as mybir
import concourse.tile as tile
from concourse import bacc
from concourse.bass_utils import run_bass_kernel_spmd
from concourse.masks import make_identity

B, T, C, H = 2, 2048, 768, 12
D = C // H            # 64
N_CORES = 8
G = 4                 # head groups
HG = H // G           # 3 heads per group
CG = HG * D           # 192 output columns per group
NT = T // 128         # 16 t-chunks
NCC = C // 128        # 6 c-chunks
TQ = 512              # q block width
F32 = mybir.dt.float32
F32R = mybir.dt.float32r
EXP = mybir.ActivationFunctionType.Exp
SCALE = float(D) ** -0.5
SENT = 1024.0         # causal-mask sentinel: exp(scale*0 - scale*SENT) ~ 0


def _body(nc, tc, x, wkv, wp, bp, cos3, sin3, out_t, sim_variant=False):
    with tc.tile_pool(name="const", bufs=1) as cp:
        ident = cp.tile([128, 128], F32)
        make_identity(nc, ident[:])
        identR = cp.tile([128, 128], F32R)
        nc.scalar.copy(identR[:], ident[:])
        # tri[p, f] = 1.0 if f >= p else 0.0  (keep tq >= tk in diagonal blocks)
        tri = cp.tile([128, 128], F32)
        nc.gpsimd.memset(tri[:], 1.0)
        nc.gpsimd.affine_select(
            out=tri[:], in_=tri[:], compare_op=mybir.AluOpType.is_ge,
            fill=0.0, base=0, pattern=[[1, 128]], channel_multiplier=-1)

        # cdup/sdup: [128, NT*192], cols i*192 + h*64 + {2j, 2j+1} = cos/sin_j
        # (DMAs issued inside the stage-1 loop so x chunk 0 loads first)
        cos_sb = cp.tile([128, NT * 192], F32)
        sin_sb = cp.tile([128, NT * 192], F32)
        # weights feed f32r matmuls: DRAM-side bitcast (PE rounds internally)
        wkv_sb = cp.tile([128, NCC * 2 * CG], F32R)
        nc.sync.dma_start(wkv_sb[:].rearrange("p (n m) -> p n m", n=NCC),
                          wkv.rearrange("(n p) m -> p n m", p=128).bitcast(F32R))
        wp_sb = cp.tile([128, NCC * CG], F32R)
        bp_sb = cp.tile([1, CG], F32R)
        ones_f = cp.tile([1, TQ], F32)
        nc.gpsimd.memset(ones_f[:], 1.0)
        ones_sb = cp.tile([1, TQ], F32R)
        nc.scalar.copy(ones_sb[:], ones_f[:])
        # bias AP for masked-diagonal exp: exp(scale*s' - scale*SENT)
        nbias = cp.tile([128, 1], F32)
        nc.gpsimd.memset(nbias[:], -SENT * SCALE)

        # persistent per-head [D, T] tensors: heads 0,1 packed in one tile
        kT01 = cp.tile([128, T], F32R)
        kT2 = cp.tile([64, T], F32R)
        qT01 = cp.tile([128, T], F32R)
        qT2 = cp.tile([64, T], F32R)
        # V in [T, D] layout with a ones column appended per head: per
        # t-chunk i, cols [i*195 + h*65 : .. + 64] = V_h, col .. + 64 = 1.0
        vaug = cp.tile([128, NT * (HG * 65)], F32R)
        ones48 = cp.tile([128, NT * HG], F32)
        nc.gpsimd.memset(ones48[:], 1.0)
        nc.scalar.copy(
            vaug[:].rearrange("p (k c) -> p k c", c=65)[:, :, 64], ones48[:])
        # attention output, transposed [CG, T]
        oT01 = cp.tile([128, T], F32)
        oT2 = cp.tile([64, T], F32)

        # ---- Stage 1: KV projection + RoPE + transposes ----
        with tc.tile_pool(name="s1", bufs=3) as s1, \
             tc.tile_pool(name="s1ps", bufs=2, space="PSUM") as s1ps:
            for i in range(NT):
                x_sb = s1.tile([128, C], F32R, tag="x")
                nc.sync.dma_start(x_sb[:],
                                  x[i * 128:(i + 1) * 128, :].bitcast(F32R))
                if i == 1:
                    # rope tables load while chunk 0 is transposing
                    nc.sync.dma_start(cos_sb[:], cos3)
                    nc.sync.dma_start(sin_sb[:], sin3)
                # batched PE transpose of the whole [128, C] chunk
                xtp = s1ps.tile([128, C], F32R, tag="xtp", bufs=1)
                for c in range(NCC):
                    nc.tensor.transpose(xtp[:, c * 128:(c + 1) * 128],
                                        x_sb[:, c * 128:(c + 1) * 128],
                                        identR[:])
                xT_sb = s1.tile([128, C], F32R, tag="xTs")
                nc.scalar.copy(xT_sb[:], xtp[:])
                kv_ps = s1ps.tile([128, 2 * CG], F32, tag="kv")
                for c in range(NCC):
                    nc.tensor.matmul(
                        kv_ps[:], xT_sb[:, c * 128:(c + 1) * 128],
                        wkv_sb[:, c * 2 * CG:(c + 1) * 2 * CG],
                        start=(c == 0), stop=(c == NCC - 1))
                # SBUF staging of K|V (ACT): rope reads then hit the DVE
                # fp32 SBUF fast path, and vaug copies from SBUF too
                kv_sb = s1.tile([128, 2 * CG], F32, tag="kvs")
                nc.scalar.copy(kv_sb[:], kv_ps[:])

                # RoPE (4 DVE ops per half): K half -> kq[:, 0:CG],
                # Q = rope(V) half -> kq[:, CG:2CG]
                kq_sb = s1.tile([128, 2 * CG], F32R, tag="kq")
                cS = cos_sb[:, i * CG:(i + 1) * CG]
                sS = sin_sb[:, i * CG:(i + 1) * CG]
                for off in (0, CG):
                    kvh = kv_sb[:, off:off + CG]
                    a_sb = s1.tile([128, CG], F32, tag="ra")
                    b_sb = s1.tile([128, CG], F32, tag="rb")
                    nc.vector.tensor_mul(a_sb[:], kvh, cS)
                    nc.vector.tensor_mul(b_sb[:], kvh, sS)
                    nc.vector.tensor_sub(kq_sb[:, off:off + CG:2],
                                         a_sb[:, 0:CG:2], b_sb[:, 1:CG:2])
                    nc.vector.tensor_add(kq_sb[:, off + 1:off + CG:2],
                                         b_sb[:, 0:CG:2], a_sb[:, 1:CG:2])

                # V (unroped) into vaug [T, 65*3] layout
                vdst = vaug[:, i * 195:(i + 1) * 195] \
                    .rearrange("p (h c) -> p h c", h=HG)[:, :, 0:64]
                vsrc = kv_sb[:, CG:2 * CG].rearrange("p (h c) -> p h c", h=HG)
                nc.vector.tensor_copy(vdst, vsrc)

                # transpose roped K and Q into [D, T] per-head layouts;
                # all 4 transposes share one single-bank PSUM tile
                tp = s1ps.tile([128, 4 * 128], F32R, tag="tp")
                nc.tensor.transpose(tp[:, 0:128], kq_sb[:, 0:128], identR[:])
                nc.tensor.transpose(tp[0:64, 128:256], kq_sb[:, 128:192],
                                    identR[:])
                nc.tensor.transpose(tp[:, 256:384], kq_sb[:, 192:320],
                                    identR[:])
                nc.tensor.transpose(tp[0:64, 384:512], kq_sb[:, 320:384],
                                    identR[:])
                ts_ = slice(i * 128, (i + 1) * 128)
                nc.scalar.copy(kT01[:, ts_], tp[:, 0:128])
                nc.vector.tensor_copy(kT2[:, ts_], tp[0:64, 128:256])
                nc.scalar.copy(qT01[:, ts_], tp[:, 256:384])
                nc.vector.tensor_copy(qT2[:, ts_], tp[0:64, 384:512])

        # projection weights load during stage 2
        nc.sync.dma_start(wp_sb[:].rearrange("p (n m) -> p n m", n=NCC),
                          wp.rearrange("(n p) m -> p n m", p=128).bitcast(F32R))
        nc.sync.dma_start(bp_sb[:], bp.bitcast(F32R))

        # ---- Stage 2: causal attention, transposed score layout ----
        # b-outer so each T-half of oT completes early for the split AG
        with tc.tile_pool(name="s2", bufs=6) as s2, \
             tc.tile_pool(name="s2ps", bufs=2, space="PSUM") as s2ps:
            for b in range(T // TQ):
                for h in range(HG):
                    kT = (kT01[0:64], kT01[64:128], kT2[0:64])[h]
                    qT = (qT01[0:64], qT01[64:128], qT2[0:64])[h]
                    oT = (oT01[0:64], oT01[64:128], oT2[0:64])[h]
                    bs = b * TQ
                    nblk = (bs + TQ) // 128
                    o_ps = s2ps.tile([65, TQ], F32, tag="o", bufs=3)
                    for t in range(nblk):
                        diag = t * 128 >= bs
                        col0 = t * 128 - bs if diag else 0
                        ncols = TQ - col0
                        s_ps = s2ps.tile([128, TQ], F32, tag="s", bufs=4)
                        nc.tensor.matmul(
                            s_ps[:, 0:ncols], kT[:, t * 128:(t + 1) * 128],
                            qT[:, bs + col0:bs + TQ], start=True, stop=True)
                        wei = s2.tile([128, TQ], F32R, tag="wei")
                        if diag:
                            # mask tq<tk: s' = (s + SENT)*tri, then biased exp
                            nc.vector.scalar_tensor_tensor(
                                s_ps[:, 0:128], s_ps[:, 0:128], SENT, tri[:],
                                mybir.AluOpType.add, mybir.AluOpType.mult)
                            nc.scalar.activation(wei[:, 0:128], s_ps[:, 0:128],
                                                 EXP, scale=SCALE,
                                                 bias=nbias[:])
                            if ncols > 128:
                                nc.scalar.activation(wei[:, 128:ncols],
                                                     s_ps[:, 128:ncols],
                                                     EXP, scale=SCALE)
                        else:
                            nc.scalar.activation(wei[:, 0:ncols],
                                                 s_ps[:, 0:ncols],
                                                 EXP, scale=SCALE)
                        va = t * 195 + h * 65
                        nc.tensor.matmul(
                            o_ps[:, col0:TQ], vaug[:, va:va + 65],
                            wei[:, 0:ncols],
                            start=(t == 0), stop=(t == nblk - 1))
                    recip = s2.tile([1, TQ], F32, tag="recip")
                    nc.vector.reciprocal(recip[:], o_ps[64:65, :])
                    rb = s2.tile([64, TQ], F32, tag="rb")
                    nc.gpsimd.partition_broadcast(rb[:], recip[:])
                    nc.vector.tensor_mul(oT[:, bs:bs + TQ], o_ps[0:64, :], rb[:])

        # ---- Stage 3: split AllGather within batch group + projection ----
        TH = T // 2
        with tc.tile_pool(name="s3", bufs=3) as s3, \
             tc.tile_pool(name="s3ps", bufs=2, space="PSUM") as s3ps, \
             tc.tile_pool(name="dram", bufs=1, space="DRAM") as dp:
            for half in range(2):
                hs = slice(half * TH, (half + 1) * TH)
                ag_in = dp.tile([CG, TH], F32, tag=f"agi{half}")
                ag_out = dp.tile([G * CG, TH], F32, tag=f"ago{half}")
                nc.sync.dma_start(ag_in[0:128, :], oT01[:, hs])
                nc.sync.dma_start(ag_in[128:CG, :], oT2[:, hs])
                if sim_variant:
                    # collective-free stand-in for TimelineSim: approximate
                    # the AllGather with DRAM copies of the same byte volume
                    for gg in range(G):
                        nc.sync.dma_start(ag_out[gg * CG:(gg + 1) * CG, :],
                                          ag_in[:])
                else:
                    nc.gpsimd.collective_compute(
                        "AllGather", mybir.AluOpType.bypass,
                        replica_groups=[[0, 1, 2, 3], [4, 5, 6, 7]],
                        ins=[ag_in[:].opt()], outs=[ag_out[:].opt()])
                # whole gathered half into SBUF in one DMA
                a_all = s3.tile([128, NCC * TH], F32R, tag="aall")
                nc.sync.dma_start(
                    a_all[:].rearrange("p (n m) -> p n m", n=NCC),
                    ag_out[:].rearrange("(n p) m -> p n m", p=128)
                    .bitcast(F32R))
                o_all0 = s3.tile([128, TH], F32, tag="oall0")
                o_all1 = s3.tile([64, TH], F32, tag="oall1")
                for j2 in range(TH // TQ):
                    j = half * (TH // TQ) + j2
                    p0 = s3ps.tile([128, TQ], F32, tag="p0")
                    p1 = s3ps.tile([64, TQ], F32, tag="p1")
                    for c in range(NCC):
                        a_sb = a_all[:, c * TH + j2 * TQ:c * TH + (j2 + 1) * TQ]
                        nc.tensor.matmul(p0[:], wp_sb[:, c * CG:c * CG + 128],
                                         a_sb, start=(c == 0), stop=False)
                        nc.tensor.matmul(p1[:],
                                         wp_sb[:, c * CG + 128:(c + 1) * CG],
                                         a_sb, start=(c == 0), stop=False)
                    nc.tensor.matmul(p0[:], bp_sb[:, 0:128], ones_sb[:],
                                     start=False, stop=True)
                    nc.tensor.matmul(p1[:], bp_sb[:, 128:CG], ones_sb[:],
                                     start=False, stop=True)
                    js2 = slice(j2 * TQ, (j2 + 1) * TQ)
                    nc.vector.tensor_copy(o_all0[:, js2], p0[:])
                    nc.vector.tensor_copy(o_all1[:, js2], p1[:])
                hs2 = slice(half * TH, (half + 1) * TH)
                nc.sync.dma_start(out_t[0:128, hs2], o_all0[:])
                nc.scalar.dma_start(out_t[128:CG, hs2], o_all1[:])


def _build(sim_variant=False):
    nc = bacc.Bacc("TRN2", target_bir_lowering=False, debug=False,
                   num_devices=1 if sim_variant else N_CORES,
                   enable_asserts=False)
    x = nc.dram_tensor("x", [T, C], F32, kind="ExternalInput").ap()
    wkv = nc.dram_tensor("wkv", [C, 2 * CG], F32, kind="ExternalInput").ap()
    wp = nc.dram_tensor("wp", [C, CG], F32, kind="ExternalInput").ap()
    bp = nc.dram_tensor("bp", [1, CG], F32, kind="ExternalInput").ap()
    cos3 = nc.dram_tensor("cos3", [128, NT * 192], F32, kind="ExternalInput").ap()
    sin3 = nc.dram_tensor("sin3", [128, NT * 192], F32, kind="ExternalInput").ap()
    out_t = nc.dram_tensor("out_t", [CG, T], F32, kind="ExternalOutput").ap()
    with tile.TileContext(nc) as tc:
        _body(nc, tc, x, wkv, wp, bp, cos3, sin3, out_t, sim_variant)
    nc.compile()
    return nc


_NC = None


def _get_nc():
    global _NC
    if _NC is None:
        _NC = _build()
    return _NC


_EXEC = None


def _get_exec():
    """Reusable jitted SPMD executable (mirrors bass2jax.run_bass_via_pjrt's
    multi-core path, without donation: our kernel writes every output
    element, so the pre-zeroed output buffers can be device-resident
    constants reused across calls)."""
    global _EXEC
    if _EXEC is not None:
        return _EXEC
    import jax
    from jax.experimental.shard_map import shard_map
    from jax.sharding import Mesh, PartitionSpec
    from concourse import bass2jax, mybir as _mybir

    nc = _get_nc()
    bass2jax.install_neuronx_cc_hook()
    in_names, out_names, out_avals, zero_outs = [], [], [], []
    assert nc.dbg_addr is None
    pname = nc.partition_id_tensor.name if nc.partition_id_tensor else None
    for alloc in nc.m.functions[0].allocations:
        if not isinstance(alloc, _mybir.MemoryLocationSet):
            continue
        name = alloc.memorylocations[0].name
        if alloc.kind == "ExternalInput":
            if name != pname:
                in_names.append(name)
        elif alloc.kind == "ExternalOutput":
            out_names.append(name)
            shape = tuple(alloc.tensor_shape)
            dtype = _mybir.dt.np(alloc.dtype)
            out_avals.append(jax.core.ShapedArray(shape, dtype))
            zero_outs.append(np.zeros(shape, dtype))
    n_params = len(in_names)
    all_names = in_names + out_names
    if pname is not None:
        all_names = all_names + [pname]

    def _fn(*args):
        operands = list(args)
        if pname is not None:
            operands.append(bass2jax.partition_id_tensor())
        outs = bass2jax._bass_exec_p.bind(
            *operands,
            out_avals=tuple(out_avals),
            in_names=tuple(all_names),
            out_names=tuple(out_names),
            lowering_input_output_aliases=(),
            sim_require_finite=True,
            sim_require_nnan=True,
            nc=nc,
        )
        return tuple(outs)

    devices = jax.devices()[:N_CORES]
    mesh = Mesh(np.asarray(devices), ("core",))
    nin = n_params + len(out_names)
    donate = tuple(range(n_params, n_params + len(out_names)))
    sharded = jax.jit(
        shard_map(_fn, mesh=mesh,
                  in_specs=(PartitionSpec("core"),) * nin,
                  out_specs=(PartitionSpec("core"),) * len(out_names),
                  check_rep=False),
        donate_argnums=donate, keep_unused=True)

    def _zero_cat():
        return [np.zeros((N_CORES * z.shape[0], *z.shape[1:]), z.dtype)
                for z in zero_outs]

    _EXEC = (sharded, in_names, out_names, out_avals, _zero_cat)
    return _EXEC


def _run_cached(in_maps):
    sharded, in_names, out_names, out_avals, zero_cat = _get_exec()
    concat_in = [np.concatenate([np.asarray(in_maps[c][n])
                                 for c in range(N_CORES)], axis=0)
                 for n in in_names]
    out_arrs = sharded(*concat_in, *zero_cat())
    return [
        {name: np.asarray(out_arrs[i]).reshape(N_CORES, *out_avals[i].shape)[c]
         for i, name in enumerate(out_names)}
        for c in range(N_CORES)
    ]


def _prep_rope(r):
    # [T, 32] -> [128, NT*192]: chunk i cols [i*192 + h*64 + 2j, +2j+1] both
    # hold r[i*128+p, j] (duplicated across the channel pair, per head)
    rr = r.reshape(NT, 128, 32).transpose(1, 0, 2)           # [128, NT, 32]
    rr = np.repeat(rr, 2, axis=2)                            # [128, NT, 64]
    rr = np.broadcast_to(rr[:, :, None, :], (128, NT, HG, 64))
    return np.ascontiguousarray(rr.reshape(128, NT * 192), dtype=np.float32)


def kernel(x, rope_cos, rope_sin, W_att, W_proj, b_proj, _run_kwargs=None):
    x = np.ascontiguousarray(np.asarray(x, np.float32))
    W_att = np.asarray(W_att, np.float32)
    W_proj = np.asarray(W_proj, np.float32)
    b_proj = np.asarray(b_proj, np.float32)
    cos3 = _prep_rope(np.asarray(rope_cos, np.float32))
    sin3 = _prep_rope(np.asarray(rope_sin, np.float32))

    nc = _get_nc()
    in_maps = []
    for r in range(N_CORES):
        b, g = divmod(r, G)
        c0, c1 = g * CG, (g + 1) * CG
        wkv = np.ascontiguousarray(
            np.concatenate([W_att[:, c0:c1],
                            W_att[:, 2 * C + c0:2 * C + c1]], axis=1))
        in_maps.append({
            "x": x[b],
            "wkv": wkv,
            "wp": np.ascontiguousarray(W_proj[:, c0:c1]),
            "bp": np.ascontiguousarray(b_proj[c0:c1][None, :]),
            "cos3": cos3,
            "sin3": sin3,
        })
    global _FIRST_CALL_DONE, _last_in_maps
    _last_in_maps = in_maps
    if not _FIRST_CALL_DONE:
        res = run_bass_kernel_spmd(nc, in_maps, core_ids=list(range(N_CORES)),
                                   **(_run_kwargs or {}))
        results = res.results
        kernel.last_results = res
        _FIRST_CALL_DONE = True
    else:
        results = _run_cached(in_maps)
    out = np.empty((B, T, C), np.float32)
    for r in range(N_CORES):
        b, g = divmod(r, G)
        out[b, :, g * CG:(g + 1) * CG] = results[r]["out_t"].T
    return out


_FIRST_CALL_DONE = False
